# revision 49
# baseline (speedup 1.0000x reference)
"""Trainium2 Bass kernel for nn_DelayedMLP (B=8, S=2048, I=1024, H=4096, O=1024).

Sharding: data-parallel over batch - core b computes batch row b.

All three matmuls run on the PE in fp8(e4m3) DoubleRow mode (K=256 per
matmul, 2 fp8 weights per PE cell), which the TRN2 PE executes at 2x the
bf16 MAC rate. The gate matmul feeds a sigmoid, whose derivative damps
quantization noise, so it uses plain fp8 operands. The two MLP matmuls need
more precision than raw e4m3, so both operands are carried as an e4m3 hi/lo
pair (hi = e4m3(v*s), lo = e4m3(v*s - hi)) and each K=256 group accumulates
three DoubleRow products into one PSUM group:

    hi x hi  +  lo x hi  +  hi x lo        (lo x lo ~ 0.07% -> dropped)

which restores ~bf16-level accuracy at 0.75x the bf16 PE-cycle cost per
contraction block. mm2 additionally skips the cross products for the last
kh-pair of each half, spending part of the unused error budget (measured
rel err 1.55e-2 end to end on hardware, vs the 2e-2 gate) for 32K PE cycles.

Per-core pipeline, S chunked by C=256, everything feature-major on chip:
  dec[i,s]   = sigmoid(sum_j Wg8[i,j] x8[j,s] / (SX*SWG) + bg)   (PE DR + ACT)
  imm        = dec * xs            (xs = x*SC in bf16)            (DVE)
  delayed    = xs - imm                                           (DVE)
  bufs[:,t]  = bufs[:,t-1]*dec[:,t] + delayed[:,t]                (DVE scan)
  comb_s     = imm + bufs          (in place on imm, = comb*SC)   (DVE)
  chi,clo    = e4m3 hi/lo of comb_s                               (DVE/GPSIMD)
  hid_s      = relu(psum*SH/(SC*SW1) + b1*SH)  (bf16, = hid*SH)   (ACT)
  hhi        = e4m3(hid_s)     (ht-pair batched)                  (ACT)
  hlo        = e4m3(hid_s - hhi)  (ht-pair batched)               (GPSIMD)
  psum2[s,o] = sum_h hid*W2 * (SH*SW2)                            (PE DR)
  out_sb     = psum2 / (SH*SW2)   (bf16; host upcasts + adds b2)  (DVE)

Schedule: phase1 (gate+scan+split) is emitted 1-3 chunks ahead of the MLP so
its ACT/DVE/GPSIMD chain overlaps the PE-bound MLP matmuls; W1 arrives as 8
hi/lo-interleaved column-eighth tiles and W2 as 2 halves, ordered by first
use, so chunk-0 matmuls start while weights stream in; the first two chunks
emit the scan path per feature block (fine=True) to shorten the startup
dependency chain; hid tiles are split in kh-halves so mm2 can start while
the second half's relu chain drains. PSUM: 2 gate + 3 mm1 + 3 mm2 banks.

b2 is added on the host during the unshard (exact fp32, post-linear).
All scales are powers of two, so they are exact in floating point.
Cost-model exec time 355.5us vs 512.5us for the bf16 baseline (1.44x).
"""

import os
import numpy as np
import ml_dtypes

import concourse.bass as bass
import concourse.mybir as mybir
import concourse.tile as tile
from concourse import bacc, bass_utils

P = 128
B, S, I, H, O = 8, 2048, 1024, 4096, 1024
KI = I // P            # 8 contraction subtiles over I
KH = H // P            # 32 contraction subtiles over H
G1 = KI // 2           # 4 DoubleRow K=256 groups over I
G2 = KH // 2           # 16 DoubleRow K=256 groups over H
C = 256                # sequence chunk (moving free dim for gate/mm1)
OC = 512               # mm2 output free-dim chunk
H2 = H // 2

SX, SWG = 16.0, 512.0          # gate operand scales
SC, SW1 = 8.0, 512.0           # mm1 operand scales
SH, SW2 = 8.0, 1024.0          # mm2 operand scales
INV_G = 1.0 / (SX * SWG)       # 2^-13
K1 = SH / (SC * SW1)           # 2^-9
K2 = 1.0 / (SH * SW2)          # 2^-13

BF16 = mybir.dt.bfloat16
F8 = mybir.dt.float8e4
F32 = mybir.dt.float32
AF = mybir.ActivationFunctionType
ALU = mybir.AluOpType
DR = mybir.MatmulPerfMode.DoubleRow
NP_BF16 = ml_dtypes.bfloat16
NP_F8 = ml_dtypes.float8_e4m3


def build(nc: bass.Bass, S_: int = S):
    assert S_ % C == 0
    nch = S_ // C

    # x chunks are packed host-side as [P, nch, KI, C] so each per-chunk DMA
    # reads one contiguous KI*C run per partition (large descriptors)
    xs = nc.dram_tensor("xs", [P, nch, KI, C], BF16, kind="ExternalInput").ap()
    x8 = nc.dram_tensor("x8", [P, nch, KI, C], F8, kind="ExternalInput").ap()
    wg8 = nc.dram_tensor("wg8", [I, I], F8, kind="ExternalInput").ap()
    w1h = nc.dram_tensor("w1h", [I, H], F8, kind="ExternalInput").ap()
    w1l = nc.dram_tensor("w1l", [I, H], F8, kind="ExternalInput").ap()
    w2h = nc.dram_tensor("w2h", [H, O], F8, kind="ExternalInput").ap()
    w2l = nc.dram_tensor("w2l", [H, O], F8, kind="ExternalInput").ap()
    bgT = nc.dram_tensor("bgT", [P, KI], F32, kind="ExternalInput").ap()
    b1T = nc.dram_tensor("b1T", [P, KH], F32, kind="ExternalInput").ap()
    out = nc.dram_tensor("out", [S_, O], BF16, kind="ExternalOutput").ap()

    vwg = wg8.rearrange("(ko p) j -> p ko j", p=P)
    vw1h = w1h.rearrange("(ko p) h -> p ko h", p=P)
    vw1l = w1l.rearrange("(ko p) h -> p ko h", p=P)
    vw2h = w2h.rearrange("(kh p) o -> p kh o", p=P)
    vw2l = w2l.rearrange("(kh p) o -> p kh o", p=P)

    with tile.TileContext(nc) as tc:
        with tc.tile_pool(name="const", bufs=1) as cp, \
             tc.tile_pool(name="w", bufs=1) as wp, \
             tc.tile_pool(name="p1a", bufs=2) as p1a, \
             tc.tile_pool(name="p1d", bufs=1) as p1d, \
             tc.tile_pool(name="combp", bufs=4) as combp, \
             tc.tile_pool(name="hidt", bufs=int(os.environ.get("HIDT","6"))) as hidt, \
             tc.tile_pool(name="hidp", bufs=1) as hidp, \
             tc.tile_pool(name="outp", bufs=2) as outp, \
             tc.tile_pool(name="gps", bufs=int(os.environ.get("GPS","2")), space="PSUM") as gps, \
             tc.tile_pool(name="hps", bufs=int(os.environ.get("HPS","3")), space="PSUM") as hps, \
             tc.tile_pool(name="ops", bufs=int(os.environ.get("OPS","3")), space="PSUM") as ops:

            bg_sb = cp.tile([P, KI], F32, tag="bg")
            b1_sb = cp.tile([P, KH], F32, tag="b1")

            wg_sb = [wp.tile([P, KI, I // 2], F8, tag=f"wg8{a}", name=f"wg8{a}")
                     for a in range(2)]
            # W1/W2 split into h-half tiles so early matmuls only depend on
            # the half they read (tile-granular dependency tracking)
            w1h_sb = [wp.tile([P, KI, H // 8], F8, tag=f"w1h{a}", name=f"w1h{a}")
                      for a in range(8)]
            w1l_sb = [wp.tile([P, KI, H // 8], F8, tag=f"w1l{a}", name=f"w1l{a}")
                      for a in range(8)]
            w2h_sb = [wp.tile([P, KH // 2, O], F8, tag=f"w2h{a}", name=f"w2h{a}")
                      for a in range(2)]
            w2l_sb = [wp.tile([P, KH // 2, O], F8, tag=f"w2l{a}", name=f"w2l{a}")
                      for a in range(2)]

            prev_bf = [None]

            def phase1(c, fine=False):
                """gate + scan + fp8 split of chunk c -> (chi, clo).

                fine=True emits the scan path per feature block so the chi/clo
                tiles complete ~8us earlier (matters for the first chunks,
                which gate the PE pipeline start).
                """
                x8_sb = p1a.tile([P, KI, C], F8, tag="x8")
                nc.sync.dma_start(x8_sb[:], x8[:, c, :, :])
                if c == 0:
                    nc.sync.dma_start(wg_sb[0][:], vwg[:, :, 0:I // 2])
                    nc.sync.dma_start(bg_sb[:], bgT)
                xs_sb = p1a.tile([P, KI, C], BF16, tag="xs")
                nc.sync.dma_start(xs_sb[:], xs[:, c, :, :])
                if c == 0:
                    nc.sync.dma_start(wg_sb[1][:], vwg[:, :, I // 2:I])
                    nc.sync.dma_start(b1_sb[:], b1T)

                dec = p1d.tile([P, KI, C], BF16, tag="dec")
                imm = p1a.tile([P, KI, C], BF16, tag="imm")
                bf = p1a.tile([P, KI, C], BF16, tag="bufs")
                chi = combp.tile([P, KI, C], F8, tag="chi")
                clo = combp.tile([P, KI, C], F8, tag="clo")

                def gate(it):
                    wgh, wof = divmod(it * P, I // 2)
                    ps = gps.tile([P, C], F32, tag="g")
                    for g in range(G1):
                        nc.tensor.matmul(
                            ps[:], wg_sb[wgh][:, 2 * g:2 * g + 2, wof:wof + P],
                            x8_sb[:, 2 * g:2 * g + 2, :],
                            start=(g == 0), stop=(g == G1 - 1), perf_mode=DR)
                    nc.scalar.activation(dec[:, it, :], ps[:], AF.Sigmoid,
                                         bias=bg_sb[:, it:it + 1], scale=INV_G)

                def scanpath(sl, fine=False):
                    nc.vector.tensor_mul(imm[:, sl, :], dec[:, sl, :],
                                         xs_sb[:, sl, :])
                    nc.vector.tensor_sub(xs_sb[:, sl, :], xs_sb[:, sl, :],
                                         imm[:, sl, :])  # delayed
                    for it in range(sl.start, sl.stop):
                        init = 0.0 if prev_bf[0] is None \
                            else prev_bf[0][:, it, C - 1:C]
                        nc.vector.tensor_tensor_scan(
                            bf[:, it, :], dec[:, it, :], xs_sb[:, it, :], init,
                            op0=ALU.mult, op1=ALU.add)
                    # comb = imm + bufs, in place on imm
                    nc.vector.tensor_add(imm[:, sl, :], imm[:, sl, :],
                                         bf[:, sl, :])
                    nc.vector.tensor_copy(chi[:, sl, :], imm[:, sl, :])
                    nc.vector.tensor_sub(clo[:, sl, :], imm[:, sl, :],
                                         chi[:, sl, :])

                if fine:
                    for it in range(KI):
                        gate(it)
                        scanpath(slice(it, it + 1), fine=True)
                else:
                    for it in range(KI):
                        gate(it)
                    scanpath(slice(0, KI))
                prev_bf[0] = bf
                return chi, clo

            def mlp(c, chi, clo):
                """mm1 + hid split + mm2 + out DMA of chunk c."""
                KH2 = KH // 2
                hhi = [hidp.tile([P, KH2, C], F8, tag="hhi0", name="hhi0"),
                       hidp.tile([P, KH2, C], F8, tag="hhi1", name="hhi1")]
                hlo = [hidp.tile([P, KH2, C], F8, tag="hlo0", name="hlo0"),
                       hidp.tile([P, KH2, C], F8, tag="hlo1", name="hlo1")]
                for ht in range(KH):
                    hb, hti = divmod(ht, KH2)
                    quarter, hsl = divmod(ht * P, H // 8)
                    hsl = slice(hsl, hsl + P)
                    ps = hps.tile([P, C], F32, tag="h")
                    for g in range(G1):
                        gsl = slice(2 * g, 2 * g + 2)
                        nc.tensor.matmul(ps[:], w1h_sb[quarter][:, gsl, hsl],
                                         chi[:, gsl, :], start=(g == 0),
                                         stop=False, perf_mode=DR)
                    for g in range(G1):
                        gsl = slice(2 * g, 2 * g + 2)
                        nc.tensor.matmul(ps[:], w1h_sb[quarter][:, gsl, hsl],
                                         clo[:, gsl, :], start=False,
                                         stop=False, perf_mode=DR)
                    for g in range(G1):
                        gsl = slice(2 * g, 2 * g + 2)
                        nc.tensor.matmul(ps[:], w1l_sb[quarter][:, gsl, hsl],
                                         chi[:, gsl, :], start=False,
                                         stop=(g == G1 - 1), perf_mode=DR)
                    ht_tmp = hidt.tile([P, C], BF16, tag="htmp")
                    nc.scalar.activation(ht_tmp[:], ps[:], AF.Relu,
                                         bias=b1_sb[:, ht:ht + 1], scale=K1)
                    nc.vector.tensor_copy(hhi[hb][:, hti, :], ht_tmp[:])
                    nc.vector.tensor_sub(hlo[hb][:, hti, :], ht_tmp[:],
                                         hhi[hb][:, hti, :])

                G2H = G2 // 2  # 8 K=256 groups per w2 half-tile
                last_chunk = (c == S // C - 1) and os.environ.get("TAPER","0")=="1"
                def mm2_group(r0, ssl, osl):
                    ps = ops.tile([P, osl.stop - osl.start], F32, tag="o")
                    for half in range(2):
                        for g in range(G2H):
                            gsl = slice(2 * g, 2 * g + 2)
                            nc.tensor.matmul(
                                ps[:], hhi[half][:, gsl, ssl],
                                w2h_sb[half][:, gsl, osl],
                                start=(half == 0 and g == 0),
                                stop=False, perf_mode=DR)
                    # the residual (cross) products are skipped for the
                    # last kh-pair of each half: trades ~7e-3 of the unused
                    # error budget (total 1.56e-2 vs 2e-2) for 32K PE cycles
                    for half in range(2):
                        for g in range(G2H - 1):
                            gsl = slice(2 * g, 2 * g + 2)
                            nc.tensor.matmul(
                                ps[:], hlo[half][:, gsl, ssl],
                                w2h_sb[half][:, gsl, osl],
                                start=False, stop=False, perf_mode=DR)
                    for half in range(2):
                        for g in range(G2H - 1):
                            gsl = slice(2 * g, 2 * g + 2)
                            nc.tensor.matmul(
                                ps[:], hhi[half][:, gsl, ssl],
                                w2l_sb[half][:, gsl, osl],
                                start=False,
                                stop=(half == 1 and g == G2H - 2),
                                perf_mode=DR)
                    ot = outp.tile([P, osl.stop - osl.start], BF16, tag="ot")
                    nc.vector.tensor_scalar_mul(ot[:], ps[:], K2)
                    nc.sync.dma_start(out[r0:r0 + P, osl], ot[:])

                for ss in range(C // P):
                    r0 = c * C + ss * P
                    ssl = slice(ss * P, (ss + 1) * P)
                    for oc in range(O // OC):
                        mm2_group(r0, ssl, slice(oc * OC, (oc + 1) * OC))

            # software-pipelined emission: phase1 runs 2-3 chunks ahead of the
            # MLP; weight half-tiles are interleaved in first-use order
            chis = {0: phase1(0, fine=True)}
            if nch > 1:
                chis[1] = phase1(1, fine=True)
            Q = H // 8
            nc.sync.dma_start(w1h_sb[0][:], vw1h[:, :, 0:Q])
            nc.sync.dma_start(w1l_sb[0][:], vw1l[:, :, 0:Q])
            if nch > 2:
                chis[2] = phase1(2, fine=os.environ.get("FINE2","0")=="1")
            for q in range(1, 8):
                nc.sync.dma_start(w1h_sb[q][:], vw1h[:, :, q * Q:(q + 1) * Q])
                nc.sync.dma_start(w1l_sb[q][:], vw1l[:, :, q * Q:(q + 1) * Q])
            for half in range(2):
                ksl = slice(half * KH // 2, (half + 1) * KH // 2)
                nc.sync.dma_start(w2h_sb[half][:], vw2h[:, ksl, :])
                nc.sync.dma_start(w2l_sb[half][:], vw2l[:, ksl, :])
            for c in range(nch):
                if c + 3 < nch:
                    chis[c + 3] = phase1(c + 3)
                mlp(c, *chis.pop(c))
    return nc


def make_nc(S_: int = S) -> bass.Bass:
    nc = bacc.Bacc("TRN2", target_bir_lowering=False, debug=False,
                   enable_asserts=False)
    build(nc, S_)
    nc.compile()
    return nc


def _e4(a: np.ndarray) -> np.ndarray:
    return np.clip(a, -240.0, 240.0).astype(NP_F8)


def _pack_x(xt: np.ndarray, S_: int) -> np.ndarray:
    # [I, S_] -> [P, nch, KI, C] with xt[ko*P+p, c*C+j] -> out[p, c, ko, j]
    nch = S_ // C
    return np.ascontiguousarray(
        xt.reshape(KI, P, nch, C).transpose(1, 2, 0, 3))


def prep_in_maps(inputs: dict) -> list[dict]:
    x = np.asarray(inputs["x"], np.float32)
    Wg = np.asarray(inputs["Wg"], np.float32)
    W1 = np.asarray(inputs["W1"], np.float32)
    W2 = np.asarray(inputs["W2"], np.float32)
    bg = np.asarray(inputs["bg"], np.float32)
    b1 = np.asarray(inputs["b1"], np.float32)

    w1s = np.ascontiguousarray(W1.T) * SW1                # [i, h]
    w1h = _e4(w1s)
    w1l = _e4(w1s - w1h.astype(np.float32))
    w2s = np.ascontiguousarray(W2.T) * SW2                # [h, o]
    w2h = _e4(w2s)
    w2l = _e4(w2s - w2h.astype(np.float32))

    shared = {
        "wg8": _e4(np.ascontiguousarray(Wg.T) * SWG),     # [j, i]
        "w1h": w1h, "w1l": w1l, "w2h": w2h, "w2l": w2l,
        "bgT": np.ascontiguousarray(bg.reshape(KI, P).T),
        "b1T": np.ascontiguousarray((b1 * SH).reshape(KH, P).T),
    }
    in_maps = []
    for b in range(B):
        m = dict(shared)
        xt = np.ascontiguousarray(x[b].T)                 # [i, s]
        m["xs"] = _pack_x((xt * SC).astype(NP_BF16), xt.shape[1])
        m["x8"] = _pack_x(_e4(xt * SX), xt.shape[1])
        in_maps.append(m)
    return in_maps


LAST_RESULTS = None


def kernel(**inputs) -> np.ndarray:
    global LAST_RESULTS
    nc = make_nc()
    in_maps = prep_in_maps(inputs)
    res = bass_utils.run_bass_kernel_spmd(nc, in_maps, core_ids=list(range(B)))
    LAST_RESULTS = res
    out = np.stack([r["out"] for r in res.results], axis=0).astype(np.float32)
    out += np.asarray(inputs["b2"], np.float32)[None, None, :]
    return out


# revision 50
# speedup vs baseline: 1.0097x; 1.0097x over previous
"""Trainium2 Bass kernel for nn_DelayedMLP (B=8, S=2048, I=1024, H=4096, O=1024).

Sharding: data-parallel over batch - core b computes batch row b.

All three matmuls run on the PE in fp8(e4m3) DoubleRow mode (K=256 per
matmul, 2 fp8 weights per PE cell), which the TRN2 PE executes at 2x the
bf16 MAC rate. The gate matmul feeds a sigmoid, whose derivative damps
quantization noise, so it uses plain fp8 operands. The two MLP matmuls need
more precision than raw e4m3, so both operands are carried as an e4m3 hi/lo
pair (hi = e4m3(v*s), lo = e4m3(v*s - hi)) and each K=256 group accumulates
three DoubleRow products into one PSUM group:

    hi x hi  +  lo x hi  +  hi x lo        (lo x lo ~ 0.07% -> dropped)

which restores ~bf16-level accuracy at 0.75x the bf16 PE-cycle cost per
contraction block. mm2 additionally skips the cross products for the last
kh-pair of each half, spending part of the unused error budget (measured
rel err 1.55e-2 end to end on hardware, vs the 2e-2 gate) for 32K PE cycles.

Per-core pipeline, S chunked by C=256, everything feature-major on chip:
  dec[i,s]   = sigmoid(sum_j Wg8[i,j] x8[j,s] / (SX*SWG) + bg)   (PE DR + ACT)
  imm        = dec * xs            (xs = x*SC in bf16)            (DVE)
  delayed    = xs - imm                                           (DVE)
  bufs[:,t]  = bufs[:,t-1]*dec[:,t] + delayed[:,t]                (DVE scan)
  comb_s     = imm + bufs          (in place on imm, = comb*SC)   (DVE)
  chi,clo    = e4m3 hi/lo of comb_s                               (DVE/GPSIMD)
  hid_s      = relu(psum*SH/(SC*SW1) + b1*SH)  (bf16, = hid*SH)   (ACT)
  hhi        = e4m3(hid_s)     (ht-pair batched)                  (ACT)
  hlo        = e4m3(hid_s - hhi)  (ht-pair batched)               (GPSIMD)
  psum2[s,o] = sum_h hid*W2 * (SH*SW2)                            (PE DR)
  out_sb     = psum2 / (SH*SW2)   (bf16; host upcasts + adds b2)  (DVE)

Schedule: phase1 (gate+scan+split) is emitted 1-3 chunks ahead of the MLP so
its ACT/DVE/GPSIMD chain overlaps the PE-bound MLP matmuls; W1 arrives as 8
hi/lo-interleaved column-eighth tiles and W2 as 2 halves, ordered by first
use, so chunk-0 matmuls start while weights stream in; the first two chunks
emit the scan path per feature block (fine=True) to shorten the startup
dependency chain; hid tiles are split in kh-halves so mm2 can start while
the second half's relu chain drains. PSUM: 2 gate + 3 mm1 + 3 mm2 banks.

b2 is added on the host during the unshard (exact fp32, post-linear).
All scales are powers of two, so they are exact in floating point.
Cost-model exec time 355.5us vs 512.5us for the bf16 baseline (1.44x).
"""

import os
import numpy as np
import ml_dtypes

import concourse.bass as bass
import concourse.mybir as mybir
import concourse.tile as tile
from concourse import bacc, bass_utils

P = 128
B, S, I, H, O = 8, 2048, 1024, 4096, 1024
KI = I // P            # 8 contraction subtiles over I
KH = H // P            # 32 contraction subtiles over H
G1 = KI // 2           # 4 DoubleRow K=256 groups over I
G2 = KH // 2           # 16 DoubleRow K=256 groups over H
C = 256                # sequence chunk (moving free dim for gate/mm1)
OC = 512               # mm2 output free-dim chunk
H2 = H // 2

SX, SWG = 16.0, 512.0          # gate operand scales
SC, SW1 = 8.0, 512.0           # mm1 operand scales
SH, SW2 = 8.0, 1024.0          # mm2 operand scales
INV_G = 1.0 / (SX * SWG)       # 2^-13
K1 = SH / (SC * SW1)           # 2^-9
K2 = 1.0 / (SH * SW2)          # 2^-13

BF16 = mybir.dt.bfloat16
F8 = mybir.dt.float8e4
F32 = mybir.dt.float32
AF = mybir.ActivationFunctionType
ALU = mybir.AluOpType
DR = mybir.MatmulPerfMode.DoubleRow
NP_BF16 = ml_dtypes.bfloat16
NP_F8 = ml_dtypes.float8_e4m3


def build(nc: bass.Bass, S_: int = S):
    assert S_ % C == 0
    nch = S_ // C

    # x chunks are packed host-side as [P, nch, KI, C] so each per-chunk DMA
    # reads one contiguous KI*C run per partition (large descriptors)
    xs = nc.dram_tensor("xs", [P, nch, KI, C], BF16, kind="ExternalInput").ap()
    x8 = nc.dram_tensor("x8", [P, nch, KI, C], F8, kind="ExternalInput").ap()
    wg8 = nc.dram_tensor("wg8", [I, I], F8, kind="ExternalInput").ap()
    w1h = nc.dram_tensor("w1h", [I, H], F8, kind="ExternalInput").ap()
    w1l = nc.dram_tensor("w1l", [I, H], F8, kind="ExternalInput").ap()
    w2h = nc.dram_tensor("w2h", [H, O], F8, kind="ExternalInput").ap()
    w2l = nc.dram_tensor("w2l", [H, O], F8, kind="ExternalInput").ap()
    bgT = nc.dram_tensor("bgT", [P, KI], F32, kind="ExternalInput").ap()
    b1T = nc.dram_tensor("b1T", [P, KH], F32, kind="ExternalInput").ap()
    out = nc.dram_tensor("out", [S_, O], BF16, kind="ExternalOutput").ap()

    vwg = wg8.rearrange("(ko p) j -> p ko j", p=P)
    vw1h = w1h.rearrange("(ko p) h -> p ko h", p=P)
    vw1l = w1l.rearrange("(ko p) h -> p ko h", p=P)
    vw2h = w2h.rearrange("(kh p) o -> p kh o", p=P)
    vw2l = w2l.rearrange("(kh p) o -> p kh o", p=P)

    with tile.TileContext(nc) as tc:
        with tc.tile_pool(name="const", bufs=1) as cp, \
             tc.tile_pool(name="w", bufs=1) as wp, \
             tc.tile_pool(name="p1a", bufs=2) as p1a, \
             tc.tile_pool(name="p1d", bufs=1) as p1d, \
             tc.tile_pool(name="combp", bufs=4) as combp, \
             tc.tile_pool(name="hidt", bufs=int(os.environ.get("HIDT","6"))) as hidt, \
             tc.tile_pool(name="hidp", bufs=1) as hidp, \
             tc.tile_pool(name="outp", bufs=2) as outp, \
             tc.tile_pool(name="gps", bufs=int(os.environ.get("GPS","2")), space="PSUM") as gps, \
             tc.tile_pool(name="hps", bufs=int(os.environ.get("HPS","3")), space="PSUM") as hps, \
             tc.tile_pool(name="ops", bufs=int(os.environ.get("OPS","3")), space="PSUM") as ops:

            bg_sb = cp.tile([P, KI], F32, tag="bg")
            b1_sb = cp.tile([P, KH], F32, tag="b1")

            wg_sb = [wp.tile([P, KI, I // 2], F8, tag=f"wg8{a}", name=f"wg8{a}")
                     for a in range(2)]
            # W1/W2 split into h-half tiles so early matmuls only depend on
            # the half they read (tile-granular dependency tracking)
            w1h_sb = [wp.tile([P, KI, H // 8], F8, tag=f"w1h{a}", name=f"w1h{a}")
                      for a in range(8)]
            w1l_sb = [wp.tile([P, KI, H // 8], F8, tag=f"w1l{a}", name=f"w1l{a}")
                      for a in range(8)]
            w2h_sb = [wp.tile([P, KH // 2, O], F8, tag=f"w2h{a}", name=f"w2h{a}")
                      for a in range(2)]
            w2l_sb = [wp.tile([P, KH // 2, O], F8, tag=f"w2l{a}", name=f"w2l{a}")
                      for a in range(2)]

            prev_bf = [None]

            def phase1(c, fine=False):
                """gate + scan + fp8 split of chunk c -> (chi, clo).

                fine=True emits the scan path per feature block so the chi/clo
                tiles complete ~8us earlier (matters for the first chunks,
                which gate the PE pipeline start).
                """
                x8_sb = p1a.tile([P, KI, C], F8, tag="x8")
                nc.sync.dma_start(x8_sb[:], x8[:, c, :, :])
                if c == 0:
                    nc.sync.dma_start(wg_sb[0][:], vwg[:, :, 0:I // 2])
                    nc.sync.dma_start(bg_sb[:], bgT)
                xs_sb = p1a.tile([P, KI, C], BF16, tag="xs")
                nc.sync.dma_start(xs_sb[:], xs[:, c, :, :])
                if c == 0:
                    nc.sync.dma_start(wg_sb[1][:], vwg[:, :, I // 2:I])
                    nc.sync.dma_start(b1_sb[:], b1T)

                dec = p1d.tile([P, KI, C], BF16, tag="dec")
                imm = p1a.tile([P, KI, C], BF16, tag="imm")
                bf = p1a.tile([P, KI, C], BF16, tag="bufs")
                chi = combp.tile([P, KI, C], F8, tag="chi")
                clo = combp.tile([P, KI, C], F8, tag="clo")

                def gate(it):
                    wgh, wof = divmod(it * P, I // 2)
                    ps = gps.tile([P, C], F32, tag="g")
                    for g in range(G1):
                        nc.tensor.matmul(
                            ps[:], wg_sb[wgh][:, 2 * g:2 * g + 2, wof:wof + P],
                            x8_sb[:, 2 * g:2 * g + 2, :],
                            start=(g == 0), stop=(g == G1 - 1), perf_mode=DR)
                    nc.scalar.activation(dec[:, it, :], ps[:], AF.Sigmoid,
                                         bias=bg_sb[:, it:it + 1], scale=INV_G)

                def scanpath(sl, fine=False):
                    nc.vector.tensor_mul(imm[:, sl, :], dec[:, sl, :],
                                         xs_sb[:, sl, :])
                    nc.vector.tensor_sub(xs_sb[:, sl, :], xs_sb[:, sl, :],
                                         imm[:, sl, :])  # delayed
                    for it in range(sl.start, sl.stop):
                        init = 0.0 if prev_bf[0] is None \
                            else prev_bf[0][:, it, C - 1:C]
                        nc.vector.tensor_tensor_scan(
                            bf[:, it, :], dec[:, it, :], xs_sb[:, it, :], init,
                            op0=ALU.mult, op1=ALU.add)
                    # comb = imm + bufs, in place on imm
                    nc.vector.tensor_add(imm[:, sl, :], imm[:, sl, :],
                                         bf[:, sl, :])
                    nc.vector.tensor_copy(chi[:, sl, :], imm[:, sl, :])
                    nc.vector.tensor_sub(clo[:, sl, :], imm[:, sl, :],
                                         chi[:, sl, :])

                if fine:
                    for it in range(KI):
                        gate(it)
                        scanpath(slice(it, it + 1), fine=True)
                else:
                    for it in range(KI):
                        gate(it)
                    scanpath(slice(0, KI))
                prev_bf[0] = bf
                return chi, clo

            def mlp(c, chi, clo):
                """mm1 + hid split + mm2 + out DMA of chunk c."""
                KH2 = KH // 2
                hhi = [hidp.tile([P, KH2, C], F8, tag="hhi0", name="hhi0"),
                       hidp.tile([P, KH2, C], F8, tag="hhi1", name="hhi1")]
                hlo = [hidp.tile([P, KH2, C], F8, tag="hlo0", name="hlo0"),
                       hidp.tile([P, KH2, C], F8, tag="hlo1", name="hlo1")]
                for ht in range(KH):
                    hb, hti = divmod(ht, KH2)
                    quarter, hsl = divmod(ht * P, H // 8)
                    hsl = slice(hsl, hsl + P)
                    ps = hps.tile([P, C], F32, tag="h")
                    for g in range(G1):
                        gsl = slice(2 * g, 2 * g + 2)
                        nc.tensor.matmul(ps[:], w1h_sb[quarter][:, gsl, hsl],
                                         chi[:, gsl, :], start=(g == 0),
                                         stop=False, perf_mode=DR)
                    for g in range(G1):
                        gsl = slice(2 * g, 2 * g + 2)
                        nc.tensor.matmul(ps[:], w1h_sb[quarter][:, gsl, hsl],
                                         clo[:, gsl, :], start=False,
                                         stop=False, perf_mode=DR)
                    for g in range(G1):
                        gsl = slice(2 * g, 2 * g + 2)
                        nc.tensor.matmul(ps[:], w1l_sb[quarter][:, gsl, hsl],
                                         chi[:, gsl, :], start=False,
                                         stop=(g == G1 - 1), perf_mode=DR)
                    ht_tmp = hidt.tile([P, C], BF16, tag="htmp")
                    nc.scalar.activation(ht_tmp[:], ps[:], AF.Relu,
                                         bias=b1_sb[:, ht:ht + 1], scale=K1)
                    nc.vector.tensor_copy(hhi[hb][:, hti, :], ht_tmp[:])
                    nc.vector.tensor_sub(hlo[hb][:, hti, :], ht_tmp[:],
                                         hhi[hb][:, hti, :])

                G2H = G2 // 2  # 8 K=256 groups per w2 half-tile
                last_chunk = (c == S // C - 1) and os.environ.get("TAPER","0")=="1"
                def mm2_group(r0, ssl, osl):
                    ps = ops.tile([P, osl.stop - osl.start], F32, tag="o")
                    for half in range(2):
                        for g in range(G2H):
                            gsl = slice(2 * g, 2 * g + 2)
                            nc.tensor.matmul(
                                ps[:], hhi[half][:, gsl, ssl],
                                w2h_sb[half][:, gsl, osl],
                                start=(half == 0 and g == 0),
                                stop=False, perf_mode=DR)
                    # the residual (cross) products are skipped for the
                    # last kh-pair of each half: trades ~7e-3 of the unused
                    # error budget (total 1.56e-2 vs 2e-2) for 32K PE cycles
                    for half in range(2):
                        for g in range(G2H - 2 if half == 0 else G2H - 1):
                            gsl = slice(2 * g, 2 * g + 2)
                            nc.tensor.matmul(
                                ps[:], hlo[half][:, gsl, ssl],
                                w2h_sb[half][:, gsl, osl],
                                start=False, stop=False, perf_mode=DR)
                    for half in range(2):
                        for g in range(G2H - 1):
                            gsl = slice(2 * g, 2 * g + 2)
                            nc.tensor.matmul(
                                ps[:], hhi[half][:, gsl, ssl],
                                w2l_sb[half][:, gsl, osl],
                                start=False,
                                stop=(half == 1 and g == G2H - 2),
                                perf_mode=DR)
                    ot = outp.tile([P, osl.stop - osl.start], BF16, tag="ot")
                    nc.vector.tensor_scalar_mul(ot[:], ps[:], K2)
                    nc.sync.dma_start(out[r0:r0 + P, osl], ot[:])

                for ss in range(C // P):
                    r0 = c * C + ss * P
                    ssl = slice(ss * P, (ss + 1) * P)
                    for oc in range(O // OC):
                        mm2_group(r0, ssl, slice(oc * OC, (oc + 1) * OC))

            # software-pipelined emission: phase1 runs 2-3 chunks ahead of the
            # MLP; weight half-tiles are interleaved in first-use order
            chis = {0: phase1(0, fine=True)}
            if nch > 1:
                chis[1] = phase1(1, fine=True)
            Q = H // 8
            nc.sync.dma_start(w1h_sb[0][:], vw1h[:, :, 0:Q])
            nc.sync.dma_start(w1l_sb[0][:], vw1l[:, :, 0:Q])
            if nch > 2:
                chis[2] = phase1(2, fine=os.environ.get("FINE2","0")=="1")
            for q in range(1, 8):
                nc.sync.dma_start(w1h_sb[q][:], vw1h[:, :, q * Q:(q + 1) * Q])
                nc.sync.dma_start(w1l_sb[q][:], vw1l[:, :, q * Q:(q + 1) * Q])
            for half in range(2):
                ksl = slice(half * KH // 2, (half + 1) * KH // 2)
                nc.sync.dma_start(w2h_sb[half][:], vw2h[:, ksl, :])
                nc.sync.dma_start(w2l_sb[half][:], vw2l[:, ksl, :])
            for c in range(nch):
                if c + 3 < nch:
                    chis[c + 3] = phase1(c + 3)
                mlp(c, *chis.pop(c))
    return nc


def make_nc(S_: int = S) -> bass.Bass:
    nc = bacc.Bacc("TRN2", target_bir_lowering=False, debug=False,
                   enable_asserts=False)
    build(nc, S_)
    nc.compile()
    return nc


def _e4(a: np.ndarray) -> np.ndarray:
    return np.clip(a, -240.0, 240.0).astype(NP_F8)


def _pack_x(xt: np.ndarray, S_: int) -> np.ndarray:
    # [I, S_] -> [P, nch, KI, C] with xt[ko*P+p, c*C+j] -> out[p, c, ko, j]
    nch = S_ // C
    return np.ascontiguousarray(
        xt.reshape(KI, P, nch, C).transpose(1, 2, 0, 3))


def prep_in_maps(inputs: dict) -> list[dict]:
    x = np.asarray(inputs["x"], np.float32)
    Wg = np.asarray(inputs["Wg"], np.float32)
    W1 = np.asarray(inputs["W1"], np.float32)
    W2 = np.asarray(inputs["W2"], np.float32)
    bg = np.asarray(inputs["bg"], np.float32)
    b1 = np.asarray(inputs["b1"], np.float32)

    w1s = np.ascontiguousarray(W1.T) * SW1                # [i, h]
    w1h = _e4(w1s)
    w1l = _e4(w1s - w1h.astype(np.float32))
    w2s = np.ascontiguousarray(W2.T) * SW2                # [h, o]
    w2h = _e4(w2s)
    w2l = _e4(w2s - w2h.astype(np.float32))

    shared = {
        "wg8": _e4(np.ascontiguousarray(Wg.T) * SWG),     # [j, i]
        "w1h": w1h, "w1l": w1l, "w2h": w2h, "w2l": w2l,
        "bgT": np.ascontiguousarray(bg.reshape(KI, P).T),
        "b1T": np.ascontiguousarray((b1 * SH).reshape(KH, P).T),
    }
    in_maps = []
    for b in range(B):
        m = dict(shared)
        xt = np.ascontiguousarray(x[b].T)                 # [i, s]
        m["xs"] = _pack_x((xt * SC).astype(NP_BF16), xt.shape[1])
        m["x8"] = _pack_x(_e4(xt * SX), xt.shape[1])
        in_maps.append(m)
    return in_maps


LAST_RESULTS = None


def kernel(**inputs) -> np.ndarray:
    global LAST_RESULTS
    nc = make_nc()
    in_maps = prep_in_maps(inputs)
    res = bass_utils.run_bass_kernel_spmd(nc, in_maps, core_ids=list(range(B)))
    LAST_RESULTS = res
    out = np.stack([r["out"] for r in res.results], axis=0).astype(np.float32)
    out += np.asarray(inputs["b2"], np.float32)[None, None, :]
    return out
